# revision 1
# baseline (speedup 1.0000x reference)
"""BigBird attention kernel for 8 Trainium2 NeuronCores.

Sharding: data-parallel over batch (2) x tensor-parallel over heads (4 groups
of 4 heads) = 8 cores. Each core computes q/k/v projections for its head
slice, block-sparse masked attention (128x128 supertiles derived from the
runtime mask), and a partial output projection with its Wo row-slice. The
host sums the 4 partial outputs per batch.
"""

import sys

for _p in ("/opt/trn_rl_repo", "/opt/trn_rl_repo/concourse"):
    if _p not in sys.path:
        sys.path.insert(0, _p)

import numpy as np

import concourse.bacc as bacc
import concourse.bass as bass
import concourse.mybir as mybir
import concourse.tile as tile
from concourse import bass_utils

F32 = mybir.dt.float32
F32R = mybir.dt.float32r
BF16 = mybir.dt.bfloat16

B, S, D, H = 2, 2048, 1024, 16
HD = D // H          # 64
SCALE = 1.0 / float(np.sqrt(HD))
NCORES = 8
HG = 4               # head groups (tensor-parallel)
HPC = H // HG        # heads per core = 4
DC = HPC * HD        # channels per core = 256
QT = 128             # supertile edge
NQ = S // QT         # 16
NK = S // QT         # 16
VW = HD + 1          # v columns per head incl. ones column


def _mask_pattern(mask):
    """Derive the block-sparse schedule from the runtime mask."""
    sup = mask.reshape(NQ, QT, NK, QT).any(axis=(1, 3))  # [16,16]
    kts = [np.nonzero(sup[qi])[0].tolist() for qi in range(NQ)]
    cnts = [len(k) for k in kts]
    maxw = max(max(cnts), 1) * QT
    # segments: consecutive kt runs, split so each scores matmul stays inside
    # one psum bank (4 slots of 128 = 512 fp32, the fp32 moving-N limit too)
    segs = []
    for qi in range(NQ):
        s = []
        slot = 0
        while slot < cnts[qi]:
            start = slot
            while (
                slot + 1 < cnts[qi]
                and kts[qi][slot + 1] == kts[qi][slot] + 1
                and (slot + 1) // 4 == start // 4
            ):
                slot += 1
            s.append((start, kts[qi][start], slot - start + 1))
            slot += 1
        segs.append(s)
    return kts, cnts, segs, maxw


def _build_nc(kts, cnts, segs, maxw):
    nc = bacc.Bacc("TRN2", target_bir_lowering=False, debug=False)

    xT_d = nc.dram_tensor("xT", [D, S], F32R, kind="ExternalInput")
    wq_d = nc.dram_tensor("wq", [D, DC], F32R, kind="ExternalInput")
    wk_d = nc.dram_tensor("wk", [D, DC], F32R, kind="ExternalInput")
    wv_d = nc.dram_tensor("wv", [D, DC], F32R, kind="ExternalInput")
    wo_d = nc.dram_tensor("wo", [DC, D], F32R, kind="ExternalInput")
    cos_d = nc.dram_tensor("cosT", [128, S], F32, kind="ExternalInput")
    sin_d = nc.dram_tensor("sinT", [128, S], F32, kind="ExternalInput")
    rt_d = nc.dram_tensor("rT", [128, 128], F32R, kind="ExternalInput")
    id_d = nc.dram_tensor("ident", [128, 128], F32R, kind="ExternalInput")
    mk_d = nc.dram_tensor("maskc", [NQ, QT, maxw], BF16, kind="ExternalInput")
    out_d = nc.dram_tensor("out", [S, D], F32, kind="ExternalOutput")

    KC = D // 128   # 8 contraction chunks
    CC = DC // 128  # 2 channel chunks (2 heads each)

    with tile.TileContext(nc) as tc:
        with (
            tc.tile_pool(name="persist", bufs=1) as pp,
        ):
            # persistent sbuf tensors
            qrT = [pp.tile([128, S], F32R, tag=f"qrT{c}", name=f"qrT{c}") for c in range(CC)]
            krT = [pp.tile([128, S], F32R, tag=f"krT{c}", name=f"krT{c}") for c in range(CC)]
            v_sb = [pp.tile([128, DC], F32R, tag=f"v{i}", name=f"v{i}") for i in range(NQ)]
            ident = pp.tile([128, 128], F32R, tag="ident")
            nc.sync.dma_start(ident[:], id_d[:, :])

            # ---------------- QKV + RoPE ----------------
            with (
                tc.tile_pool(name="qkv_in", bufs=1) as qp,
                tc.tile_pool(name="qkv_scr", bufs=4) as sp,
                tc.tile_pool(name="qkv_ps", bufs=2, space="PSUM") as psp,
                tc.tile_pool(name="qkv_psv", bufs=2, space="PSUM") as psv,
            ):
                xT = [qp.tile([128, S], F32R, tag=f"xT{k}", name=f"xT{k}") for k in range(KC)]
                wq_sb = [qp.tile([128, DC], F32R, tag=f"wq{k}", name=f"wq{k}") for k in range(KC)]
                wk_sb = [qp.tile([128, DC], F32R, tag=f"wk{k}", name=f"wk{k}") for k in range(KC)]
                wv_sb = [qp.tile([128, DC], F32R, tag=f"wv{k}", name=f"wv{k}") for k in range(KC)]
                cosT = qp.tile([128, S], F32, tag="cosT")
                sinT = qp.tile([128, S], F32, tag="sinT")
                rT = qp.tile([128, 128], F32R, tag="rT")
                for k in range(KC):
                    nc.sync.dma_start(xT[k][:], xT_d[k * 128:(k + 1) * 128, :])
                    nc.sync.dma_start(wq_sb[k][:], wq_d[k * 128:(k + 1) * 128, :])
                    nc.sync.dma_start(wk_sb[k][:], wk_d[k * 128:(k + 1) * 128, :])
                    nc.sync.dma_start(wv_sb[k][:], wv_d[k * 128:(k + 1) * 128, :])
                nc.sync.dma_start(cosT[:], cos_d[:, :])
                nc.sync.dma_start(sinT[:], sin_d[:, :])
                nc.sync.dma_start(rT[:], rt_d[:, :])

                # q^T / k^T with rope applied in-place
                for cc in range(CC):
                    for pc in range(S // 512):
                        fs = slice(pc * 512, (pc + 1) * 512)
                        for w_sb, dstT, tg in (
                            (wq_sb, qrT, "q"),
                            (wk_sb, krT, "k"),
                        ):
                            ps = psp.tile([128, 512], F32, tag=f"ps_{tg}", name=f"ps_{tg}")
                            for k in range(KC):
                                nc.tensor.matmul(
                                    ps[:],
                                    w_sb[k][:, cc * 128:(cc + 1) * 128],
                                    xT[k][:, fs],
                                    start=(k == 0),
                                    stop=(k == KC - 1),
                                )
                            raw = sp.tile([128, 512], F32R, tag="raw")
                            nc.scalar.copy(raw[:], ps[:])
                            rot = psp.tile([128, 512], F32, tag="rot")
                            nc.tensor.matmul(
                                rot[:], rT[:], raw[:], start=True, stop=True
                            )
                            u = sp.tile([128, 512], F32, tag="u")
                            nc.vector.tensor_mul(u[:], rot[:], sinT[:, fs])
                            nc.vector.tensor_mul(dstT[cc][:, fs], raw[:], cosT[:, fs])
                            nc.vector.tensor_add(
                                dstT[cc][:, fs], dstT[cc][:, fs], u[:]
                            )

                # v natural, packed [128, 4*65] with a ones column per head
                for pi in range(NQ):
                    ps_v = psv.tile([128, DC], F32, tag="ps_v")
                    for k in range(KC):
                        nc.tensor.matmul(
                            ps_v[:],
                            xT[k][:, pi * 128:(pi + 1) * 128],
                            wv_sb[k][:],
                            start=(k == 0),
                            stop=(k == KC - 1),
                        )
                    nc.scalar.copy(v_sb[pi][:], ps_v[:])

            # ---------------- attention ----------------
            with tc.tile_pool(name="otp", bufs=1) as otp:
              with (
                tc.tile_pool(name="at_m", bufs=8) as mp,
                  tc.tile_pool(name="at_p", bufs=4) as ep,
                  tc.tile_pool(name="at_pt", bufs=6) as tp,
                  tc.tile_pool(name="at_sc", bufs=8) as scp,
                  tc.tile_pool(name="ps_s", bufs=4, space="PSUM") as pss,
                  tc.tile_pool(name="ps_o", bufs=2, space="PSUM") as pso,
                  tc.tile_pool(name="ps_t", bufs=2, space="PSUM") as pst,
              ):
                  otT = [otp.tile([128, S], F32R, tag=f"otT{c}", name=f"otT{c}")
                         for c in range(CC)]
                  wo_sb = [otp.tile([128, D], F32R, tag=f"wo{c}", name=f"wo{c}")
                           for c in range(CC)]
                  for c in range(CC):
                      nc.sync.dma_start(wo_sb[c][:], wo_d[c * 128:(c + 1) * 128, :])
                  for qig in range(NQ // 4):
                      qis = list(range(4 * qig, 4 * qig + 4))
                      mks = {}
                      for qi in qis:
                          mks[qi] = mp.tile([128, maxw], BF16, tag="mk", name="mk")
                          nc.sync.dma_start(
                              mks[qi][:, :cnts[qi] * QT], mk_d[qi, :, :cnts[qi] * QT]
                          )
                      for h in range(HPC):
                          cc, ho = h // 2, (h % 2) * 64
                          po4 = pso.tile([64, 512], F32, tag="po4")
                          for qi in qis:
                              if cnts[qi] == 0:
                                  # fully-masked q tile: zero contribution
                                  nc.vector.memset(
                                      po4[:, (qi % 4) * 128:(qi % 4 + 1) * 128],
                                      0.0,
                                  )
                                  continue
                              w = cnts[qi] * QT
                              nbank = (cnts[qi] + 3) // 4
                              qs = slice(qi * 128, (qi + 1) * 128)
                              ps_b = [
                                  pss.tile([128, 512], F32, tag="ps_s", name="ps_s")
                                  for _ in range(nbank)
                              ]
                              for slot, kt0, ln in segs[qi]:
                                  off = (slot % 4) * 128
                                  nc.tensor.matmul(
                                      ps_b[slot // 4][:, off:off + ln * 128],
                                      qrT[cc][ho:ho + 64, qs],
                                      krT[cc][ho:ho + 64, kt0 * 128:(kt0 + ln) * 128],
                                      start=True,
                                      stop=True,
                                  )
                              pe = ep.tile([128, maxw], F32, tag="pe")
                              for bi in range(nbank):
                                  wb = min(w - bi * 512, 512)
                                  nc.scalar.activation(
                                      pe[:, bi * 512:bi * 512 + wb],
                                      ps_b[bi][:, :wb],
                                      mybir.ActivationFunctionType.Exp,
                                      bias=0.0,
                                      scale=SCALE,
                                  )
                              # masked P + row-sum in one DVE pass; the softmax
                              # division rides along in the transposing matmul
                              # via a diag(1/l) moving operand
                              l = scp.tile([128, 1], F32, tag="l")
                              peR = ep.tile([128, maxw], F32R, tag="peR")
                              nc.vector.scalar_tensor_tensor(
                                  peR[:, :w], pe[:, :w], 1.0, mks[qi][:, :w],
                                  mybir.AluOpType.mult, mybir.AluOpType.mult,
                                  accum_out=l[:],
                              )
                              r = scp.tile([128, 1], F32, tag="r")
                              nc.vector.reciprocal(r[:], l[:])
                              dg = scp.tile([128, 128], F32R, tag="dg")
                              nc.scalar.mul(dg[:], ident[:], r[:])
                              ptb = tp.tile([128, maxw], F32R, tag="ptb")
                              for bi in range(nbank):
                                  wb = min(w - bi * 512, 512)
                                  pt_ps = pst.tile([128, 512], F32, tag="pt_ps")
                                  for j4 in range(wb // 128):
                                      j = bi * 4 + j4
                                      nc.tensor.matmul(
                                          pt_ps[:, j4 * 128:(j4 + 1) * 128],
                                          peR[:, j * 128:(j + 1) * 128],
                                          dg[:],
                                          start=True,
                                          stop=True,
                                      )
                                  nc.vector.tensor_copy(
                                      ptb[:, bi * 512:bi * 512 + wb], pt_ps[:, :wb]
                                  )
                              for j, kt in enumerate(kts[qi]):
                                  nc.tensor.matmul(
                                      po4[:, (qi % 4) * 128:(qi % 4 + 1) * 128],
                                      v_sb[kt][:, h * HD:(h + 1) * HD],
                                      ptb[:, j * 128:(j + 1) * 128],
                                      start=(j == 0),
                                      stop=(j == cnts[qi] - 1),
                                  )
                          nc.vector.tensor_copy(
                              otT[cc][ho:ho + 64, qig * 512:(qig + 1) * 512],
                              po4[:],
                          )

              # ---------------- output projection ----------------
              with (
                  tc.tile_pool(name="wo_sc", bufs=3) as wsc,
                  tc.tile_pool(name="wo_ps", bufs=2, space="PSUM") as wps,
              ):
                  for qi in range(NQ):
                      ob = wsc.tile([128, D], F32, tag="ob")
                      for n2 in range(2):
                          pw = wps.tile([128, 512], F32, tag="pw")
                          for cc2 in range(CC):
                              nc.tensor.matmul(
                                  pw[:],
                                  otT[cc2][:, qi * 128:(qi + 1) * 128],
                                  wo_sb[cc2][:, n2 * 512:(n2 + 1) * 512],
                                  start=(cc2 == 0),
                                  stop=(cc2 == CC - 1),
                              )
                          nc.scalar.copy(ob[:, n2 * 512:(n2 + 1) * 512], pw[:])
                      nc.sync.dma_start(out_d[qi * 128:(qi + 1) * 128, :], ob[:])

    nc.compile()
    return nc


def _host_inputs(x, freqs_cos, freqs_sin, position_ids, mask01, kts, cnts, maxw,
                 Wq, Wk, Wv, Wo):
    """Per-core input maps."""
    in_maps = []
    r64 = np.zeros((HD, HD), np.float32)
    for i in range(HD // 2):
        r64[2 * i, 2 * i + 1] = -1.0
        r64[2 * i + 1, 2 * i] = 1.0
    r128 = np.zeros((128, 128), np.float32)
    r128[:64, :64] = r64
    r128[64:, 64:] = r64
    rT = np.ascontiguousarray(r128.T)
    ident = np.eye(128, dtype=np.float32)

    import ml_dtypes
    maskc = np.zeros((NQ, QT, maxw), ml_dtypes.bfloat16)
    for qi in range(NQ):
        for j, kt in enumerate(kts[qi]):
            maskc[qi, :, j * QT:(j + 1) * QT] = mask01[
                qi * QT:(qi + 1) * QT, kt * QT:(kt + 1) * QT
            ]

    for c in range(NCORES):
        b, g = c // HG, c % HG
        pos = np.clip(position_ids[b].astype(np.int64), 0, freqs_cos.shape[0] - 1)
        cos_g = np.asarray(freqs_cos)[pos]  # [S, 32]
        sin_g = np.asarray(freqs_sin)[pos]
        cosT64 = np.repeat(cos_g.T, 2, axis=0).astype(np.float32)  # [64, S]
        sinT64 = np.repeat(sin_g.T, 2, axis=0).astype(np.float32)
        cs = slice(g * DC, (g + 1) * DC)
        in_maps.append({
            "xT": np.ascontiguousarray(x[b].T).astype(np.float32),
            "wq": np.ascontiguousarray(Wq[:, cs]).astype(np.float32),
            "wk": np.ascontiguousarray(Wk[:, cs]).astype(np.float32),
            "wv": np.ascontiguousarray(Wv[:, cs]).astype(np.float32),
            "wo": np.ascontiguousarray(Wo[cs, :]).astype(np.float32),
            "cosT": np.concatenate([cosT64, cosT64], axis=0),
            "sinT": np.concatenate([sinT64, sinT64], axis=0),
            "rT": rT,
            "ident": ident,
            "maskc": maskc,
        })
    return in_maps


_CACHE = {}


def _get_nc(mask_key, kts, cnts, segs, maxw):
    if mask_key not in _CACHE:
        _CACHE[mask_key] = _build_nc(kts, cnts, segs, maxw)
    return _CACHE[mask_key]


def kernel(x, freqs_cos, freqs_sin, position_ids, bigbird_mask, Wq, Wk, Wv, Wo,
           _want_results=False, _trace=False, **trace_kwargs):
    x = np.asarray(x)
    mask = np.asarray(bigbird_mask).astype(bool)
    kts, cnts, segs, maxw = _mask_pattern(mask)
    nc = _get_nc(mask.tobytes(), kts, cnts, segs, maxw)
    in_maps = _host_inputs(
        x, np.asarray(freqs_cos), np.asarray(freqs_sin), np.asarray(position_ids),
        mask.astype(np.float32), kts, cnts, maxw,
        np.asarray(Wq), np.asarray(Wk), np.asarray(Wv), np.asarray(Wo),
    )
    res = bass_utils.run_bass_kernel_spmd(
        nc, in_maps, list(range(NCORES)), trace=_trace, **trace_kwargs
    )
    out = np.zeros((B, S, D), np.float32)
    for c in range(NCORES):
        out[c // HG] += res.results[c]["out"]
    if _want_results:
        return out, res
    return out



# revision 3
# speedup vs baseline: 1.6230x; 1.6230x over previous
"""BigBird attention kernel for 8 Trainium2 NeuronCores.

Sharding: data-parallel over batch (2) x tensor-parallel over heads (4 groups
of 4 heads) = 8 cores. Each core computes q/k/v projections for its head
slice, block-sparse masked attention over 128x128 supertiles derived from the
runtime mask, and a partial output projection with its Wo row-slice. The host
sums the 4 partial outputs per batch.

All matmuls run in bf16 (1 cycle/row + fast weight load). Scores are computed
transposed (S^T[k, q]) so the post-softmax P^T feeds attn@V directly as the
stationary operand - no per-supertile transpose matmuls. The softmax row sum
rides along as a ones-column appended to V; the division happens on the
[q, 65] attention output where q is the partition dim (native per-partition
scale), followed by one small 128x128 transpose matmul per output chunk.
"""

import sys

for _p in ("/opt/trn_rl_repo", "/opt/trn_rl_repo/concourse"):
    if _p not in sys.path:
        sys.path.insert(0, _p)

import numpy as np

import concourse.bacc as bacc
import concourse.bass as bass
import concourse.mybir as mybir
import concourse.tile as tile
from concourse import bass_utils

F32 = mybir.dt.float32
BF16 = mybir.dt.bfloat16

B, S, D, H = 2, 2048, 1024, 16
HD = D // H          # 64
SCALE = 1.0 / float(np.sqrt(HD))
NCORES = 8
HG = 4               # head groups (tensor-parallel)
HPC = H // HG        # heads per core = 4
DC = HPC * HD        # channels per core = 256
QT = 128             # supertile edge
NQ = S // QT         # 16
VW = HD + 1          # v columns per head incl. ones column (65)
KC = D // 128        # 8 contraction chunks
CC = DC // 128       # 2 channel chunks (2 heads each)


def _mask_pattern(mask):
    """Active 128x128 supertiles per q-tile from the runtime mask."""
    sup = mask.reshape(NQ, QT, NQ, QT).any(axis=(1, 3))  # [16,16]
    kts = [np.nonzero(sup[qi])[0].tolist() for qi in range(NQ)]
    cnts = [len(k) for k in kts]
    maxw = max(max(cnts), 1) * QT
    return kts, cnts, maxw


def _build_nc(kts, cnts, maxw):
    nc = bacc.Bacc("TRN2", target_bir_lowering=False, debug=False)

    xT_d = nc.dram_tensor("xT", [D, S], BF16, kind="ExternalInput")
    wq_d = nc.dram_tensor("wq", [D, DC], BF16, kind="ExternalInput")
    wk_d = nc.dram_tensor("wk", [D, DC], BF16, kind="ExternalInput")
    wv_d = nc.dram_tensor("wv", [D, HPC * VW], BF16, kind="ExternalInput")
    wo_d = nc.dram_tensor("wo", [DC, D], BF16, kind="ExternalInput")
    cos_d = nc.dram_tensor("cosT", [128, S], BF16, kind="ExternalInput")
    sin_d = nc.dram_tensor("sinT", [128, S], BF16, kind="ExternalInput")
    rt_d = nc.dram_tensor("rT", [128, 128], BF16, kind="ExternalInput")
    id_d = nc.dram_tensor("ident", [128, 128], BF16, kind="ExternalInput")
    mk_d = nc.dram_tensor("maskT", [NQ, QT, maxw], BF16, kind="ExternalInput")
    out_d = nc.dram_tensor("out", [S, D], F32, kind="ExternalOutput")

    with tile.TileContext(nc) as tc:
        with (
            tc.tile_pool(name="persist", bufs=1) as pp,
        ):
            # persistent sbuf tensors
            qrT = [pp.tile([128, S], BF16, tag=f"qrT{c}", name=f"qrT{c}") for c in range(CC)]
            krT = [pp.tile([128, S], BF16, tag=f"krT{c}", name=f"krT{c}") for c in range(CC)]
            v_sb = [pp.tile([128, HPC, VW], BF16, tag=f"v{i}", name=f"v{i}") for i in range(NQ)]
            otT = [pp.tile([128, S], BF16, tag=f"otT{c}", name=f"otT{c}") for c in range(CC)]
            wo_sb = [pp.tile([128, D], BF16, tag=f"wo{c}", name=f"wo{c}") for c in range(CC)]
            identb = pp.tile([128, 128], BF16, tag="ident")
            nc.sync.dma_start(identb[:], id_d[:, :])
            for c in range(CC):
                nc.sync.dma_start(wo_sb[c][:], wo_d[c * 128:(c + 1) * 128, :])

            # ---------------- QKV + RoPE ----------------
            with (
                tc.tile_pool(name="qkv_in", bufs=1) as qp,
                tc.tile_pool(name="qkv_scr", bufs=4) as sp,
                tc.tile_pool(name="qkv_ps", bufs=2, space="PSUM") as psp,
                tc.tile_pool(name="qkv_rot", bufs=2, space="PSUM") as psr,
                tc.tile_pool(name="qkv_psv", bufs=2, space="PSUM") as psv,
            ):
                xT = [qp.tile([128, S], BF16, tag=f"xT{k}", name=f"xT{k}") for k in range(KC)]
                wq_sb = [qp.tile([128, DC], BF16, tag=f"wq{k}", name=f"wq{k}") for k in range(KC)]
                wk_sb = [qp.tile([128, DC], BF16, tag=f"wk{k}", name=f"wk{k}") for k in range(KC)]
                wv_sb = [qp.tile([128, HPC * VW], BF16, tag=f"wv{k}", name=f"wv{k}") for k in range(KC)]
                cosT = qp.tile([128, S], BF16, tag="cosT")
                sinT = qp.tile([128, S], BF16, tag="sinT")
                rT = qp.tile([128, 128], BF16, tag="rT")
                for k in range(KC):
                    nc.sync.dma_start(xT[k][:], xT_d[k * 128:(k + 1) * 128, :])
                    nc.sync.dma_start(wq_sb[k][:], wq_d[k * 128:(k + 1) * 128, :])
                    nc.sync.dma_start(wk_sb[k][:], wk_d[k * 128:(k + 1) * 128, :])
                    nc.sync.dma_start(wv_sb[k][:], wv_d[k * 128:(k + 1) * 128, :])
                nc.sync.dma_start(cosT[:], cos_d[:, :])
                nc.sync.dma_start(sinT[:], sin_d[:, :])
                nc.sync.dma_start(rT[:], rt_d[:, :])

                # q^T / k^T with rope applied
                for cc in range(CC):
                    for pc in range(S // 512):
                        fs = slice(pc * 512, (pc + 1) * 512)
                        for w_sb, dstT, tg in (
                            (wq_sb, qrT, "q"),
                            (wk_sb, krT, "k"),
                        ):
                            ps = psp.tile([128, 512], F32, tag="ps_qk", name="ps_qk")
                            for k in range(KC):
                                nc.tensor.matmul(
                                    ps[:],
                                    w_sb[k][:, cc * 128:(cc + 1) * 128],
                                    xT[k][:, fs],
                                    start=(k == 0),
                                    stop=(k == KC - 1),
                                )
                            raw = sp.tile([128, 512], BF16, tag="raw")
                            nc.scalar.copy(raw[:], ps[:])
                            rot = psr.tile([128, 512], F32, tag="rot")
                            nc.tensor.matmul(
                                rot[:], rT[:], raw[:], start=True, stop=True
                            )
                            u = sp.tile([128, 512], BF16, tag="u")
                            nc.vector.tensor_mul(u[:], rot[:], sinT[:, fs])
                            nc.vector.tensor_mul(dstT[cc][:, fs], raw[:], cosT[:, fs])
                            nc.vector.tensor_add(
                                dstT[cc][:, fs], dstT[cc][:, fs], u[:]
                            )

                # v natural [s, 4*65] with a ones column per head
                for pi in range(NQ):
                    ps_v = psv.tile([128, 512], F32, tag="ps_v")
                    for k in range(KC):
                        nc.tensor.matmul(
                            ps_v[:, 0:HPC * VW],
                            xT[k][:, pi * 128:(pi + 1) * 128],
                            wv_sb[k][:],
                            start=(k == 0),
                            stop=(k == KC - 1),
                        )
                    nc.vector.tensor_copy(
                        v_sb[pi][:, :, :], ps_v[:, 0:HPC * VW]
                    )
                    nc.vector.memset(v_sb[pi][:, :, HD:VW], 1.0)

            # ---------------- attention + output projection ----------------
            with (
                tc.tile_pool(name="at_m", bufs=3) as mp,
                tc.tile_pool(name="at_p", bufs=3) as ep,
                tc.tile_pool(name="at_sc", bufs=2) as scp,
                tc.tile_pool(name="at_ob", bufs=2) as obp,
                tc.tile_pool(name="ps_s", bufs=2, space="PSUM") as pss,
                tc.tile_pool(name="ps_o", bufs=2, space="PSUM") as pso,
                tc.tile_pool(name="ps_t", bufs=1, space="PSUM") as pst,
                tc.tile_pool(name="ps_w", bufs=1, space="PSUM") as psw,
            ):
                for qi in range(NQ):
                    cnt = cnts[qi]
                    w = cnt * QT
                    mk = mp.tile([128, maxw], BF16, tag="mk", name="mk")
                    nc.sync.dma_start(mk[:, :w], mk_d[qi, :, :w])
                    po = pso.tile([128, HPC, 128], F32, tag="po", name="po")
                    for h in range(HPC):
                        cc, ho = h // 2, (h % 2) * 64
                        qs = slice(qi * 128, (qi + 1) * 128)
                        # scores^T[k, q] per supertile
                        ps_sc = pss.tile([128, maxw], F32, tag="ps_sc", name="ps_sc")
                        for j, kt in enumerate(kts[qi]):
                            nc.tensor.matmul(
                                ps_sc[:, j * 128:(j + 1) * 128],
                                krT[cc][ho:ho + 64, kt * 128:(kt + 1) * 128],
                                qrT[cc][ho:ho + 64, qs],
                                start=True,
                                stop=True,
                            )
                        # exp -> bf16, then mask multiply
                        pT = ep.tile([128, maxw], BF16, tag="pT", name="pT")
                        nc.scalar.activation(
                            pT[:, :w],
                            ps_sc[:, :w],
                            mybir.ActivationFunctionType.Exp,
                            bias=0.0,
                            scale=SCALE,
                        )
                        nc.vector.tensor_mul(pT[:, :w], pT[:, :w], mk[:, :w])
                        # attn @ V' (ones column gives the softmax row sums)
                        for j, kt in enumerate(kts[qi]):
                            nc.tensor.matmul(
                                po[:, h:h + 1, 0:VW],
                                pT[:, j * 128:(j + 1) * 128],
                                v_sb[kt][:, h:h + 1, :],
                                start=(j == 0),
                                stop=(j == cnt - 1),
                            )
                    # softmax division (per-partition scale on natural-q layout)
                    r = scp.tile([128, HPC], F32, tag="r", name="r")
                    nc.vector.reciprocal(r[:, :], po[:, :, HD:VW])
                    o_nat = scp.tile([128, DC], BF16, tag="o_nat", name="o_nat")
                    for h in range(HPC):
                        nc.scalar.mul(
                            o_nat[:, h * HD:(h + 1) * HD],
                            po[:, h:h + 1, 0:HD],
                            r[:, h:h + 1],
                        )
                    # transpose O[q, c] -> otT[c, q]
                    ot_ps = pst.tile([128, 512], F32, tag="ot_ps", name="ot_ps")
                    for c2 in range(CC):
                        nc.tensor.matmul(
                            ot_ps[:, c2 * 128:(c2 + 1) * 128],
                            o_nat[:, c2 * 128:(c2 + 1) * 128],
                            identb[:],
                            start=True,
                            stop=True,
                        )
                    for c2 in range(CC):
                        nc.vector.tensor_copy(
                            otT[c2][:, qs], ot_ps[:, c2 * 128:(c2 + 1) * 128]
                        )
                    # output projection for this q chunk
                    ob = obp.tile([128, D], F32, tag="ob", name="ob")
                    for n2 in range(2):
                        pw = psw.tile([128, 512], F32, tag="pw", name="pw")
                        for cc2 in range(CC):
                            nc.tensor.matmul(
                                pw[:],
                                otT[cc2][:, qs],
                                wo_sb[cc2][:, n2 * 512:(n2 + 1) * 512],
                                start=(cc2 == 0),
                                stop=(cc2 == CC - 1),
                            )
                        if n2 == 0:
                            nc.vector.tensor_copy(ob[:, 0:512], pw[:])
                        else:
                            nc.scalar.copy(ob[:, 512:1024], pw[:])
                    nc.sync.dma_start(out_d[qi * 128:(qi + 1) * 128, :], ob[:])

    nc.compile()
    return nc


def _host_inputs(x, freqs_cos, freqs_sin, position_ids, mask01, kts, cnts, maxw,
                 Wq, Wk, Wv, Wo):
    """Per-core input maps."""
    import ml_dtypes
    bf = ml_dtypes.bfloat16

    in_maps = []
    r64 = np.zeros((HD, HD), np.float32)
    for i in range(HD // 2):
        r64[2 * i, 2 * i + 1] = -1.0
        r64[2 * i + 1, 2 * i] = 1.0
    r128 = np.zeros((128, 128), np.float32)
    r128[:64, :64] = r64
    r128[64:, 64:] = r64
    rT = np.ascontiguousarray(r128.T).astype(bf)
    ident = np.eye(128, dtype=np.float32).astype(bf)

    # transposed mask supertiles packed per q-tile: [k_local, j*128 + q_local]
    maskT = np.zeros((NQ, QT, maxw), bf)
    for qi in range(NQ):
        for j, kt in enumerate(kts[qi]):
            maskT[qi, :, j * QT:(j + 1) * QT] = mask01[
                qi * QT:(qi + 1) * QT, kt * QT:(kt + 1) * QT
            ].T

    for c in range(NCORES):
        b, g = c // HG, c % HG
        pos = np.clip(position_ids[b].astype(np.int64), 0, freqs_cos.shape[0] - 1)
        cos_g = np.asarray(freqs_cos)[pos]  # [S, 32]
        sin_g = np.asarray(freqs_sin)[pos]
        cosT64 = np.repeat(cos_g.T, 2, axis=0).astype(np.float32)  # [64, S]
        sinT64 = np.repeat(sin_g.T, 2, axis=0).astype(np.float32)
        cs = slice(g * DC, (g + 1) * DC)
        wv_g = np.asarray(Wv)[:, cs].astype(np.float32)  # [D, 256]
        wv260 = np.zeros((D, HPC * VW), np.float32)
        for h in range(HPC):
            wv260[:, h * VW:h * VW + HD] = wv_g[:, h * HD:(h + 1) * HD]
        in_maps.append({
            "xT": np.ascontiguousarray(x[b].T).astype(bf),
            "wq": np.ascontiguousarray(Wq[:, cs]).astype(bf),
            "wk": np.ascontiguousarray(Wk[:, cs]).astype(bf),
            "wv": wv260.astype(bf),
            "wo": np.ascontiguousarray(Wo[cs, :]).astype(bf),
            "cosT": np.concatenate([cosT64, cosT64], axis=0).astype(bf),
            "sinT": np.concatenate([sinT64, sinT64], axis=0).astype(bf),
            "rT": rT,
            "ident": ident,
            "maskT": maskT,
        })
    return in_maps


_CACHE = {}


def _get_nc(mask_key, kts, cnts, maxw):
    if mask_key not in _CACHE:
        _CACHE[mask_key] = _build_nc(kts, cnts, maxw)
    return _CACHE[mask_key]


def kernel(x, freqs_cos, freqs_sin, position_ids, bigbird_mask, Wq, Wk, Wv, Wo,
           _want_results=False, _trace=False, **trace_kwargs):
    x = np.asarray(x)
    mask = np.asarray(bigbird_mask).astype(bool)
    kts, cnts, maxw = _mask_pattern(mask)
    nc = _get_nc(mask.tobytes(), kts, cnts, maxw)
    in_maps = _host_inputs(
        x, np.asarray(freqs_cos), np.asarray(freqs_sin), np.asarray(position_ids),
        mask.astype(np.float32), kts, cnts, maxw,
        np.asarray(Wq), np.asarray(Wk), np.asarray(Wv), np.asarray(Wo),
    )
    res = bass_utils.run_bass_kernel_spmd(
        nc, in_maps, list(range(NCORES)), trace=_trace, **trace_kwargs
    )
    out = np.zeros((B, S, D), np.float32)
    for c in range(NCORES):
        out[c // HG] += res.results[c]["out"]
    if _want_results:
        return out, res
    return out


# revision 5
# speedup vs baseline: 1.6425x; 1.0120x over previous
"""BigBird attention kernel for 8 Trainium2 NeuronCores.

Sharding: data-parallel over batch (2) x tensor-parallel over heads (4 groups
of 4 heads) = 8 cores. Each core computes q/k/v projections for its head
slice, block-sparse masked attention over 128x128 supertiles derived from the
runtime mask, and a partial output projection with its Wo row-slice. The host
sums the 4 partial bf16 outputs per batch in fp32.

All matmuls run in bf16 (1 cycle/row). Scores are computed transposed
(S^T[k, q]) so the post-softmax P^T feeds attn@V directly as the stationary
operand - no per-supertile transpose matmuls. Head pairs within a 128-row
chunk run as concurrent row-tiled matmuls (tile_position (0,0)/(64,0)). The
softmax row sum rides as a ones-column appended to V; division happens on the
[q, 65] attention output (q = partition dim, native per-partition scale),
then one 128x128 transpose matmul per output chunk feeds the Wo projection.
"""

import sys

for _p in ("/opt/trn_rl_repo", "/opt/trn_rl_repo/concourse"):
    if _p not in sys.path:
        sys.path.insert(0, _p)

import numpy as np

import concourse.bacc as bacc
import concourse.bass as bass
import concourse.mybir as mybir
import concourse.tile as tile
from concourse import bass_utils

F32 = mybir.dt.float32
BF16 = mybir.dt.bfloat16

B, S, D, H = 2, 2048, 1024, 16
HD = D // H          # 64
SCALE = 1.0 / float(np.sqrt(HD))
NCORES = 8
HG = 4               # head groups (tensor-parallel)
HPC = H // HG        # heads per core = 4
DC = HPC * HD        # channels per core = 256
QT = 128             # supertile edge
NQ = S // QT         # 16
VW = HD + 1          # v columns per head incl. ones column (65)
KC = D // 128        # 8 contraction chunks
CC = DC // 128       # 2 channel chunks (2 heads each)


def _mask_pattern(mask):
    """Active 128x128 supertiles per q-tile from the runtime mask."""
    sup = mask.reshape(NQ, QT, NQ, QT).any(axis=(1, 3))  # [16,16]
    kts = [np.nonzero(sup[qi])[0].tolist() for qi in range(NQ)]
    cnts = [len(k) for k in kts]
    maxw = max(max(cnts), 1) * QT
    return kts, cnts, maxw


def _build_nc(kts, cnts, maxw):
    nc = bacc.Bacc("TRN2", target_bir_lowering=False, debug=False)

    xT_d = nc.dram_tensor("xT", [128, KC, S], BF16, kind="ExternalInput")
    wq_d = nc.dram_tensor("wq", [128, KC, DC], BF16, kind="ExternalInput")
    wk_d = nc.dram_tensor("wk", [128, KC, DC], BF16, kind="ExternalInput")
    wv_d = nc.dram_tensor("wv", [128, KC, HPC * VW], BF16, kind="ExternalInput")
    wo_d = nc.dram_tensor("wo", [128, CC, D], BF16, kind="ExternalInput")
    cos_d = nc.dram_tensor("cosT", [128, S], BF16, kind="ExternalInput")
    sin_d = nc.dram_tensor("sinT", [128, S], BF16, kind="ExternalInput")
    rt_d = nc.dram_tensor("rT", [128, 128], BF16, kind="ExternalInput")
    id_d = nc.dram_tensor("ident", [128, 128], BF16, kind="ExternalInput")
    mk_d = nc.dram_tensor("maskT", [128, NQ, maxw], BF16, kind="ExternalInput")
    out_d = nc.dram_tensor("out", [S, D], BF16, kind="ExternalOutput")

    with tile.TileContext(nc) as tc:
        with (
            tc.tile_pool(name="persist", bufs=1) as pp,
        ):
            # persistent sbuf tensors
            qrT = [pp.tile([128, S], BF16, tag=f"qrT{c}", name=f"qrT{c}") for c in range(CC)]
            krT = [pp.tile([128, S], BF16, tag=f"krT{c}", name=f"krT{c}") for c in range(CC)]
            v_sb = [pp.tile([128, HPC, VW], BF16, tag=f"v{i}", name=f"v{i}") for i in range(NQ)]
            otT = pp.tile([128, CC, S], BF16, tag="otT", name="otT")
            wo_sb = pp.tile([128, CC, D], BF16, tag="wo", name="wo")
            identb = pp.tile([128, 128], BF16, tag="ident")
            mk_sb = pp.tile([128, NQ, maxw], BF16, tag="maskT", name="maskT")
            nc.sync.dma_start(identb[:], id_d[:, :])
            nc.sync.dma_start(wo_sb[:], wo_d[:, :, :])
            nc.sync.dma_start(mk_sb[:], mk_d[:, :, :])

            # ---------------- QKV + RoPE ----------------
            with (
                tc.tile_pool(name="qkv_in", bufs=1) as qp,
                tc.tile_pool(name="qkv_scr", bufs=4) as sp,
                tc.tile_pool(name="qkv_ps", bufs=2, space="PSUM") as psp,
                tc.tile_pool(name="qkv_rot", bufs=2, space="PSUM") as psr,
                tc.tile_pool(name="qkv_psv", bufs=2, space="PSUM") as psv,
            ):
                wq_sb = qp.tile([128, KC, DC], BF16, tag="wq", name="wq")
                wk_sb = qp.tile([128, KC, DC], BF16, tag="wk", name="wk")
                wv_sb = qp.tile([128, KC, HPC * VW], BF16, tag="wv", name="wv")
                cosT = qp.tile([128, S], BF16, tag="cosT")
                sinT = qp.tile([128, S], BF16, tag="sinT")
                rT = qp.tile([128, 128], BF16, tag="rT")
                xT = qp.tile([128, KC, S], BF16, tag="xT", name="xT")
                nc.sync.dma_start(wq_sb[:], wq_d[:, :, :])
                nc.sync.dma_start(wk_sb[:], wk_d[:, :, :])
                nc.sync.dma_start(wv_sb[:], wv_d[:, :, :])
                nc.sync.dma_start(cosT[:], cos_d[:, :])
                nc.sync.dma_start(sinT[:], sin_d[:, :])
                nc.sync.dma_start(rT[:], rt_d[:, :])
                for pc in range(4):
                    fs = slice(pc * 512, (pc + 1) * 512)
                    nc.sync.dma_start(xT[:, :, fs], xT_d[:, :, fs])

                # q^T / k^T with rope applied
                for pc in range(S // 512):
                    fs = slice(pc * 512, (pc + 1) * 512)
                    for cc in range(CC):
                        for w_sb, dstT in (
                            (wq_sb, qrT),
                            (wk_sb, krT),
                        ):
                            ps = psp.tile([128, 512], F32, tag="ps_qk", name="ps_qk")
                            for k in range(KC):
                                nc.tensor.matmul(
                                    ps[:],
                                    w_sb[:, k, cc * 128:(cc + 1) * 128],
                                    xT[:, k, fs],
                                    start=(k == 0),
                                    stop=(k == KC - 1),
                                )
                            raw = sp.tile([128, 512], BF16, tag="raw")
                            nc.scalar.copy(raw[:], ps[:])
                            rot = psr.tile([128, 512], F32, tag="rot")
                            nc.tensor.matmul(
                                rot[:], rT[:], raw[:], start=True, stop=True
                            )
                            u = sp.tile([128, 512], BF16, tag="u")
                            nc.vector.tensor_mul(u[:], rot[:], sinT[:, fs])
                            nc.vector.tensor_mul(dstT[cc][:, fs], raw[:], cosT[:, fs])
                            nc.vector.tensor_add(
                                dstT[cc][:, fs], dstT[cc][:, fs], u[:]
                            )

                # v natural [s, 4*65] with a ones column per head
                for pi in range(NQ):
                    ps_v = psv.tile([128, 512], F32, tag="ps_v")
                    for k in range(KC):
                        nc.tensor.matmul(
                            ps_v[:, 0:HPC * VW],
                            xT[:, k, pi * 128:(pi + 1) * 128],
                            wv_sb[:, k, :],
                            start=(k == 0),
                            stop=(k == KC - 1),
                        )
                    nc.vector.tensor_copy(
                        v_sb[pi][:, :, :], ps_v[:, 0:HPC * VW]
                    )
                    nc.vector.memset(v_sb[pi][:, :, HD:VW], 1.0)

            # ---------------- attention + output projection ----------------
            with (
                tc.tile_pool(name="at_p", bufs=3) as ep,
                tc.tile_pool(name="at_sc", bufs=2) as scp,
                tc.tile_pool(name="at_ob", bufs=2) as obp,
                tc.tile_pool(name="ps_s", bufs=2, space="PSUM") as pss,
                tc.tile_pool(name="ps_o", bufs=2, space="PSUM") as pso,
                tc.tile_pool(name="ps_t", bufs=1, space="PSUM") as pst,
                tc.tile_pool(name="ps_w", bufs=1, space="PSUM") as psw,
            ):
                for qi in range(NQ):
                    cnt = cnts[qi]
                    w = cnt * QT
                    qs = slice(qi * 128, (qi + 1) * 128)
                    po = pso.tile([128, HPC, 128], F32, tag="po", name="po")
                    for cc in range(CC):
                        # paired heads 2cc / 2cc+1: concurrent row-tiled scores
                        ps_pair = [
                            pss.tile([128, maxw], F32, tag="ps_sc", name="ps_sc")
                            for _ in range(2)
                        ]
                        for j, kt in enumerate(kts[qi]):
                            for hh in range(2):
                                ho = hh * 64
                                nc.tensor.matmul(
                                    ps_pair[hh][:, j * 128:(j + 1) * 128],
                                    krT[cc][ho:ho + 64, kt * 128:(kt + 1) * 128],
                                    qrT[cc][ho:ho + 64, qs],
                                    start=True,
                                    stop=True,
                                )
                        for hh in range(2):
                            h = 2 * cc + hh
                            # exp -> bf16, then mask multiply
                            pT = ep.tile([128, maxw], BF16, tag="pT", name="pT")
                            nc.scalar.activation(
                                pT[:, :w],
                                ps_pair[hh][:, :w],
                                mybir.ActivationFunctionType.Exp,
                                bias=0.0,
                                scale=SCALE,
                            )
                            pTm = ep.tile([128, maxw], BF16, tag="pTm", name="pTm")
                            nc.vector.tensor_mul(
                                pTm[:, :w], pT[:, :w], mk_sb[:, qi, :w]
                            )
                            # attn @ V' (ones column gives softmax row sums)
                            for j, kt in enumerate(kts[qi]):
                                nc.tensor.matmul(
                                    po[:, h:h + 1, 0:VW],
                                    pTm[:, j * 128:(j + 1) * 128],
                                    v_sb[kt][:, h:h + 1, :],
                                    start=(j == 0),
                                    stop=(j == cnt - 1),
                                )
                    # softmax division (per-partition scale on natural-q layout)
                    r = scp.tile([128, HPC], F32, tag="r", name="r")
                    nc.vector.reciprocal(r[:, :], po[:, :, HD:VW])
                    o_nat = scp.tile([128, DC], BF16, tag="o_nat", name="o_nat")
                    for h in range(HPC):
                        nc.vector.tensor_scalar_mul(
                            o_nat[:, h * HD:(h + 1) * HD],
                            po[:, h:h + 1, 0:HD],
                            r[:, h:h + 1],
                        )
                    # transpose O[q, c] -> otT[c, q]
                    ot_ps = pst.tile([128, 256], F32, tag="ot_ps", name="ot_ps")
                    for c2 in range(CC):
                        nc.tensor.matmul(
                            ot_ps[:, c2 * 128:(c2 + 1) * 128],
                            o_nat[:, c2 * 128:(c2 + 1) * 128],
                            identb[:],
                            start=True,
                            stop=True,
                        )
                    nc.vector.tensor_copy(otT[:, :, qs], ot_ps[:, 0:256])
                    # output projection for this q chunk
                    ob = obp.tile([128, D], BF16, tag="ob", name="ob")
                    for n2 in range(2):
                        pw = psw.tile([128, 512], F32, tag="pw", name="pw")
                        for cc2 in range(CC):
                            nc.tensor.matmul(
                                pw[:],
                                otT[:, cc2, qs],
                                wo_sb[:, cc2, n2 * 512:(n2 + 1) * 512],
                                start=(cc2 == 0),
                                stop=(cc2 == CC - 1),
                            )
                        if n2 == 0:
                            nc.vector.tensor_copy(ob[:, 0:512], pw[:])
                        else:
                            nc.scalar.copy(ob[:, 512:1024], pw[:])
                    nc.sync.dma_start(out_d[qi * 128:(qi + 1) * 128, :], ob[:])

    nc.compile()
    return nc


def _host_inputs(x, freqs_cos, freqs_sin, position_ids, mask01, kts, cnts, maxw,
                 Wq, Wk, Wv, Wo):
    """Per-core input maps."""
    import ml_dtypes
    bf = ml_dtypes.bfloat16

    in_maps = []
    r64 = np.zeros((HD, HD), np.float32)
    for i in range(HD // 2):
        r64[2 * i, 2 * i + 1] = -1.0
        r64[2 * i + 1, 2 * i] = 1.0
    r128 = np.zeros((128, 128), np.float32)
    r128[:64, :64] = r64
    r128[64:, 64:] = r64
    rT = np.ascontiguousarray(r128.T).astype(bf)
    ident = np.eye(128, dtype=np.float32).astype(bf)

    # transposed mask supertiles packed per q-tile: [k_local, qi, j*128 + q_local]
    maskT = np.zeros((QT, NQ, maxw), bf)
    for qi in range(NQ):
        for j, kt in enumerate(kts[qi]):
            maskT[:, qi, j * QT:(j + 1) * QT] = mask01[
                qi * QT:(qi + 1) * QT, kt * QT:(kt + 1) * QT
            ].T

    def perm3(a, inner):
        # [1024, inner] -> [128, 8, inner]
        return np.ascontiguousarray(
            a.reshape(KC, 128, inner).transpose(1, 0, 2)
        ).astype(bf)

    for c in range(NCORES):
        b, g = c // HG, c % HG
        pos = np.clip(position_ids[b].astype(np.int64), 0, freqs_cos.shape[0] - 1)
        cos_g = np.asarray(freqs_cos)[pos]  # [S, 32]
        sin_g = np.asarray(freqs_sin)[pos]
        cosT64 = np.repeat(cos_g.T, 2, axis=0).astype(np.float32)  # [64, S]
        sinT64 = np.repeat(sin_g.T, 2, axis=0).astype(np.float32)
        cs = slice(g * DC, (g + 1) * DC)
        wv_g = np.asarray(Wv)[:, cs].astype(np.float32)  # [D, 256]
        wv260 = np.zeros((D, HPC * VW), np.float32)
        for h in range(HPC):
            wv260[:, h * VW:h * VW + HD] = wv_g[:, h * HD:(h + 1) * HD]
        wo_g = np.asarray(Wo)[cs, :].astype(np.float32)  # [256, 1024]
        wo3 = np.ascontiguousarray(
            wo_g.reshape(CC, 128, D).transpose(1, 0, 2)
        ).astype(bf)
        in_maps.append({
            "xT": perm3(np.ascontiguousarray(x[b].T).astype(np.float32), S),
            "wq": perm3(np.asarray(Wq)[:, cs].astype(np.float32), DC),
            "wk": perm3(np.asarray(Wk)[:, cs].astype(np.float32), DC),
            "wv": perm3(wv260, HPC * VW),
            "wo": wo3,
            "cosT": np.concatenate([cosT64, cosT64], axis=0).astype(bf),
            "sinT": np.concatenate([sinT64, sinT64], axis=0).astype(bf),
            "rT": rT,
            "ident": ident,
            "maskT": maskT,
        })
    return in_maps


_CACHE = {}


def _get_nc(mask_key, kts, cnts, maxw):
    if mask_key not in _CACHE:
        _CACHE[mask_key] = _build_nc(kts, cnts, maxw)
    return _CACHE[mask_key]


def kernel(x, freqs_cos, freqs_sin, position_ids, bigbird_mask, Wq, Wk, Wv, Wo,
           _want_results=False, _trace=False, **trace_kwargs):
    x = np.asarray(x)
    mask = np.asarray(bigbird_mask).astype(bool)
    kts, cnts, maxw = _mask_pattern(mask)
    nc = _get_nc(mask.tobytes(), kts, cnts, maxw)
    in_maps = _host_inputs(
        x, np.asarray(freqs_cos), np.asarray(freqs_sin), np.asarray(position_ids),
        mask.astype(np.float32), kts, cnts, maxw,
        np.asarray(Wq), np.asarray(Wk), np.asarray(Wv), np.asarray(Wo),
    )
    res = bass_utils.run_bass_kernel_spmd(
        nc, in_maps, list(range(NCORES)), trace=_trace, **trace_kwargs
    )
    out = np.zeros((B, S, D), np.float32)
    for c in range(NCORES):
        out[c // HG] += res.results[c]["out"].astype(np.float32)
    if _want_results:
        return out, res
    return out


# revision 13
# speedup vs baseline: 1.8279x; 1.1129x over previous
"""BigBird attention kernel for 8 Trainium2 NeuronCores.

Sharding: data-parallel over batch (2) x tensor-parallel over heads (4 groups
of 4 heads) = 8 cores. Each core computes q/k/v projections for its head
slice, block-sparse masked attention over 128x128 supertiles derived from the
runtime mask, and a partial output projection with its Wo row-slice. The host
sums the 4 partial bf16 outputs per batch in fp32.

All matmuls run in bf16 (1 cycle/row). Scores are computed transposed
(S^T[k, q]) so the post-softmax P^T feeds attn@V directly as the stationary
operand - no per-supertile transpose matmuls. Head pairs within a 128-row
chunk run as concurrent row-tiled matmuls (tile_position (0,0)/(64,0)). The
softmax row sum rides as a ones-column appended to V; division happens on the
[q, 65] attention output (q = partition dim, native per-partition scale),
then one 128x128 transpose matmul per output chunk feeds the Wo projection.
"""

import sys

for _p in ("/opt/trn_rl_repo", "/opt/trn_rl_repo/concourse"):
    if _p not in sys.path:
        sys.path.insert(0, _p)

import numpy as np

import concourse.bacc as bacc
import concourse.bass as bass
import concourse.mybir as mybir
import concourse.tile as tile
from concourse import bass_utils

F32 = mybir.dt.float32
BF16 = mybir.dt.bfloat16

B, S, D, H = 2, 2048, 1024, 16
HD = D // H          # 64
SCALE = 1.0 / float(np.sqrt(HD))
NCORES = 8
HG = 4               # head groups (tensor-parallel)
HPC = H // HG        # heads per core = 4
DC = HPC * HD        # channels per core = 256
QT = 128             # supertile edge
NQ = S // QT         # 16
VW = HD + 1          # v columns per head incl. ones column (65)
VWP = HD + 2         # padded to 66 so bf16 head blocks stay 4B-aligned
KC = D // 128        # 8 contraction chunks
CC = DC // 128       # 2 channel chunks (2 heads each)


def _mask_pattern(mask):
    """Active 128x128 supertiles per q-tile from the runtime mask."""
    sup = mask.reshape(NQ, QT, NQ, QT).any(axis=(1, 3))  # [16,16]
    kts = [np.nonzero(sup[qi])[0].tolist() for qi in range(NQ)]
    cnts = [len(k) for k in kts]
    maxw = max(max(cnts), 1) * QT
    return kts, cnts, maxw


def _build_nc(kts, cnts, maxw):
    nc = bacc.Bacc("TRN2", target_bir_lowering=False, debug=False)

    sumw = sum(cnts) * QT
    moff = [0]
    for qi in range(NQ):
        moff.append(moff[-1] + cnts[qi] * QT)

    xT_d = nc.dram_tensor("xT", [128, KC, S], BF16, kind="ExternalInput")
    wq_d = nc.dram_tensor("wq", [128, KC, DC], BF16, kind="ExternalInput")
    wk_d = nc.dram_tensor("wk", [128, KC, DC], BF16, kind="ExternalInput")
    wv_d = nc.dram_tensor("wv", [128, KC, HPC * VWP], BF16, kind="ExternalInput")
    wo_d = nc.dram_tensor("wo", [128, CC, D], BF16, kind="ExternalInput")
    cos_d = nc.dram_tensor("cosT", [128, S], BF16, kind="ExternalInput")
    sin_d = nc.dram_tensor("sinT", [128, S], BF16, kind="ExternalInput")
    rt_d = nc.dram_tensor("rT", [128, 128], BF16, kind="ExternalInput")
    id_d = nc.dram_tensor("ident", [128, 128], BF16, kind="ExternalInput")
    mk_d = nc.dram_tensor("maskT", [128, sumw], BF16, kind="ExternalInput")
    out_d = nc.dram_tensor("out", [S, D], BF16, kind="ExternalOutput")

    with tile.TileContext(nc) as tc:
        with (
            tc.tile_pool(name="persist", bufs=1) as pp,
        ):
            # persistent sbuf tensors
            qrT = [pp.tile([128, S], BF16, tag=f"qrT{c}", name=f"qrT{c}") for c in range(CC)]
            krT = [pp.tile([128, S], BF16, tag=f"krT{c}", name=f"krT{c}") for c in range(CC)]
            v_sb = [pp.tile([128, HPC, VWP], BF16, tag=f"v{i}", name=f"v{i}") for i in range(NQ)]
            otT = pp.tile([128, CC, S], BF16, tag="otT", name="otT")
            wo_sb = pp.tile([128, CC, D], BF16, tag="wo", name="wo")
            identb = pp.tile([128, 128], BF16, tag="ident")
            mk_sb = pp.tile([128, sumw], BF16, tag="maskT", name="maskT")

            # ---------------- QKV + RoPE ----------------
            with (
                tc.tile_pool(name="qkv_in", bufs=1) as qp,
                tc.tile_pool(name="qkv_scr", bufs=4) as sp,
                tc.tile_pool(name="qkv_ps", bufs=2, space="PSUM") as psp,
                tc.tile_pool(name="qkv_rot", bufs=2, space="PSUM") as psr,
                tc.tile_pool(name="qkv_psv", bufs=2, space="PSUM") as psv,
            ):
                wq_sb = qp.tile([128, KC, DC], BF16, tag="wq", name="wq")
                wk_sb = qp.tile([128, KC, DC], BF16, tag="wk", name="wk")
                wv_sb = qp.tile([128, KC, HPC * VWP], BF16, tag="wv", name="wv")
                cosT = qp.tile([128, S], BF16, tag="cosT")
                sinT = qp.tile([128, S], BF16, tag="sinT")
                rT = qp.tile([128, 128], BF16, tag="rT")
                xT = qp.tile([128, KC, S], BF16, tag="xT", name="xT")
                # DMA order = dispatch order: first-needed first
                nc.sync.dma_start(wq_sb[:], wq_d[:, :, :])
                nc.sync.dma_start(xT[:, :, 0:512], xT_d[:, :, 0:512])
                nc.sync.dma_start(wk_sb[:], wk_d[:, :, :])
                nc.sync.dma_start(cosT[:], cos_d[:, :])
                nc.sync.dma_start(sinT[:], sin_d[:, :])
                nc.sync.dma_start(rT[:], rt_d[:, :])
                for pc in range(1, 4):
                    fs = slice(pc * 512, (pc + 1) * 512)
                    nc.sync.dma_start(xT[:, :, fs], xT_d[:, :, fs])
                nc.sync.dma_start(wv_sb[:], wv_d[:, :, :])
                nc.sync.dma_start(identb[:], id_d[:, :])
                nc.sync.dma_start(wo_sb[:], wo_d[:, :, :])
                nc.sync.dma_start(mk_sb[:], mk_d[:, :])

                # q^T / k^T with rope applied
                for pc in range(S // 512):
                    fs = slice(pc * 512, (pc + 1) * 512)
                    for cc in range(CC):
                        for w_sb, dstT in (
                            (wq_sb, qrT),
                            (wk_sb, krT),
                        ):
                            ps = psp.tile([128, 512], F32, tag="ps_qk", name="ps_qk")
                            for k in range(KC):
                                nc.tensor.matmul(
                                    ps[:],
                                    w_sb[:, k, cc * 128:(cc + 1) * 128],
                                    xT[:, k, fs],
                                    start=(k == 0),
                                    stop=(k == KC - 1),
                                )
                            raw = sp.tile([128, 512], BF16, tag="raw")
                            nc.scalar.copy(raw[:], ps[:])
                            rot = psr.tile([128, 512], F32, tag="rot")
                            nc.tensor.matmul(
                                rot[:], rT[:], raw[:], start=True, stop=True
                            )
                            u = sp.tile([128, 512], BF16, tag="u")
                            nc.vector.tensor_mul(u[:], rot[:], sinT[:, fs])
                            nc.vector.tensor_mul(dstT[cc][:, fs], raw[:], cosT[:, fs])
                            nc.vector.tensor_add(
                                dstT[cc][:, fs], dstT[cc][:, fs], u[:]
                            )

                # v natural [s, 4*66] with a ones column per head
                for pi in range(NQ):
                    ps_v = psv.tile([128, 512], F32, tag="ps_v")
                    for k in range(KC):
                        nc.tensor.matmul(
                            ps_v[:, 0:HPC * VWP],
                            xT[:, k, pi * 128:(pi + 1) * 128],
                            wv_sb[:, k, :],
                            start=(k == 0),
                            stop=(k == KC - 1),
                        )
                    nc.vector.tensor_copy(
                        v_sb[pi][:, :, :], ps_v[:, 0:HPC * VWP]
                    )
                    nc.vector.memset(v_sb[pi][:, :, HD:VW], 1.0)

            # ---------------- attention + output projection ----------------
            with (
                tc.tile_pool(name="at_p", bufs=3) as ep,
                tc.tile_pool(name="at_sc", bufs=2) as scp,
                tc.tile_pool(name="at_ob", bufs=2) as obp,
                tc.tile_pool(name="ps_s", bufs=2, space="PSUM") as pss,
                tc.tile_pool(name="ps_o", bufs=2, space="PSUM") as pso,
                tc.tile_pool(name="ps_t", bufs=1, space="PSUM") as pst,
                tc.tile_pool(name="ps_w", bufs=1, space="PSUM") as psw,
            ):
                for qi in range(NQ):
                    cnt = cnts[qi]
                    w = cnt * QT
                    qs = slice(qi * 128, (qi + 1) * 128)
                    po = pso.tile([128, HPC, 128], F32, tag="po", name="po")
                    for cc in range(CC):
                        # paired heads 2cc / 2cc+1: concurrent row-tiled scores
                        ps_pair = [
                            pss.tile([128, maxw], F32, tag="ps_sc", name="ps_sc")
                            for _ in range(2)
                        ]
                        for j, kt in enumerate(kts[qi]):
                            for hh in range(2):
                                ho = hh * 64
                                nc.tensor.matmul(
                                    ps_pair[hh][:, j * 128:(j + 1) * 128],
                                    krT[cc][ho:ho + 64, kt * 128:(kt + 1) * 128],
                                    qrT[cc][ho:ho + 64, qs],
                                    start=True,
                                    stop=True,
                                )
                        for hh in range(2):
                            h = 2 * cc + hh
                            # exp -> bf16, then mask multiply
                            pT = ep.tile([128, maxw], BF16, tag="pT", name="pT")
                            nc.scalar.activation(
                                pT[:, :w],
                                ps_pair[hh][:, :w],
                                mybir.ActivationFunctionType.Exp,
                                bias=0.0,
                                scale=SCALE,
                            )
                            pTm = ep.tile([128, maxw], BF16, tag="pTm", name="pTm")
                            nc.vector.tensor_mul(
                                pTm[:, :w], pT[:, :w],
                                mk_sb[:, moff[qi]:moff[qi] + w],
                            )
                            # attn @ V' (ones column gives softmax row sums)
                            for j, kt in enumerate(kts[qi]):
                                nc.tensor.matmul(
                                    po[:, h:h + 1, 0:VW],
                                    pTm[:, j * 128:(j + 1) * 128],
                                    v_sb[kt][:, h:h + 1, 0:VW],
                                    start=(j == 0),
                                    stop=(j == cnt - 1),
                                )
                    # softmax division (per-partition scale on natural-q layout)
                    r = scp.tile([128, HPC], F32, tag="r", name="r")
                    nc.vector.reciprocal(r[:, :], po[:, :, HD:VW])
                    o_nat = scp.tile([128, DC], BF16, tag="o_nat", name="o_nat")
                    for h in range(HPC):
                        nc.vector.tensor_scalar_mul(
                            o_nat[:, h * HD:(h + 1) * HD],
                            po[:, h:h + 1, 0:HD],
                            r[:, h:h + 1],
                        )
                    # transpose O[q, c] -> otT[c, q]
                    ot_ps = pst.tile([128, 256], F32, tag="ot_ps", name="ot_ps")
                    for c2 in range(CC):
                        nc.tensor.matmul(
                            ot_ps[:, c2 * 128:(c2 + 1) * 128],
                            o_nat[:, c2 * 128:(c2 + 1) * 128],
                            identb[:],
                            start=True,
                            stop=True,
                        )
                    nc.vector.tensor_copy(otT[:, :, qs], ot_ps[:, 0:256])
                    # output projection for this q chunk
                    ob = obp.tile([128, D], BF16, tag="ob", name="ob")
                    for n2 in range(2):
                        pw = psw.tile([128, 512], F32, tag="pw", name="pw")
                        for cc2 in range(CC):
                            nc.tensor.matmul(
                                pw[:],
                                otT[:, cc2, qs],
                                wo_sb[:, cc2, n2 * 512:(n2 + 1) * 512],
                                start=(cc2 == 0),
                                stop=(cc2 == CC - 1),
                            )
                        if n2 == 0:
                            nc.vector.tensor_copy(ob[:, 0:512], pw[:])
                        else:
                            nc.scalar.copy(ob[:, 512:1024], pw[:])
                    nc.sync.dma_start(out_d[qi * 128:(qi + 1) * 128, :], ob[:])

    nc.compile()
    return nc


def _host_inputs(x, freqs_cos, freqs_sin, position_ids, mask01, kts, cnts, maxw,
                 Wq, Wk, Wv, Wo):
    """Per-core input maps."""
    import ml_dtypes
    bf = ml_dtypes.bfloat16

    in_maps = []
    r64 = np.zeros((HD, HD), np.float32)
    for i in range(HD // 2):
        r64[2 * i, 2 * i + 1] = -1.0
        r64[2 * i + 1, 2 * i] = 1.0
    r128 = np.zeros((128, 128), np.float32)
    r128[:64, :64] = r64
    r128[64:, 64:] = r64
    rT = np.ascontiguousarray(r128.T).astype(bf)
    ident = np.eye(128, dtype=np.float32).astype(bf)

    # transposed mask supertiles packed per q-tile: [k_local, off[qi] + j*128 + q_local]
    sumw = sum(cnts) * QT
    maskT = np.zeros((QT, sumw), bf)
    off = 0
    for qi in range(NQ):
        for j, kt in enumerate(kts[qi]):
            maskT[:, off + j * QT:off + (j + 1) * QT] = mask01[
                qi * QT:(qi + 1) * QT, kt * QT:(kt + 1) * QT
            ].T
        off += cnts[qi] * QT

    def perm3(a, inner):
        # [1024, inner] -> [128, 8, inner]
        return np.ascontiguousarray(
            a.reshape(KC, 128, inner).transpose(1, 0, 2)
        ).astype(bf)

    for c in range(NCORES):
        b, g = c // HG, c % HG
        pos = np.clip(position_ids[b].astype(np.int64), 0, freqs_cos.shape[0] - 1)
        cos_g = np.asarray(freqs_cos)[pos]  # [S, 32]
        sin_g = np.asarray(freqs_sin)[pos]
        cosT64 = np.repeat(cos_g.T, 2, axis=0).astype(np.float32)  # [64, S]
        sinT64 = np.repeat(sin_g.T, 2, axis=0).astype(np.float32)
        cs = slice(g * DC, (g + 1) * DC)
        wv_g = np.asarray(Wv)[:, cs].astype(np.float32)  # [D, 256]
        wv260 = np.zeros((D, HPC * VWP), np.float32)
        for h in range(HPC):
            wv260[:, h * VWP:h * VWP + HD] = wv_g[:, h * HD:(h + 1) * HD]
        wo_g = np.asarray(Wo)[cs, :].astype(np.float32)  # [256, 1024]
        wo3 = np.ascontiguousarray(
            wo_g.reshape(CC, 128, D).transpose(1, 0, 2)
        ).astype(bf)
        in_maps.append({
            "xT": perm3(np.ascontiguousarray(x[b].T).astype(np.float32), S),
            "wq": perm3(np.asarray(Wq)[:, cs].astype(np.float32), DC),
            "wk": perm3(np.asarray(Wk)[:, cs].astype(np.float32), DC),
            "wv": perm3(wv260, HPC * VWP),
            "wo": wo3,
            "cosT": np.concatenate([cosT64, cosT64], axis=0).astype(bf),
            "sinT": np.concatenate([sinT64, sinT64], axis=0).astype(bf),
            "rT": rT,
            "ident": ident,
            "maskT": maskT,
        })
    return in_maps


_CACHE = {}


def _get_nc(mask_key, kts, cnts, maxw):
    if mask_key not in _CACHE:
        _CACHE[mask_key] = _build_nc(kts, cnts, maxw)
    return _CACHE[mask_key]


def kernel(x, freqs_cos, freqs_sin, position_ids, bigbird_mask, Wq, Wk, Wv, Wo,
           _want_results=False, _trace=False, **trace_kwargs):
    x = np.asarray(x)
    mask = np.asarray(bigbird_mask).astype(bool)
    kts, cnts, maxw = _mask_pattern(mask)
    nc = _get_nc(mask.tobytes(), kts, cnts, maxw)
    in_maps = _host_inputs(
        x, np.asarray(freqs_cos), np.asarray(freqs_sin), np.asarray(position_ids),
        mask.astype(np.float32), kts, cnts, maxw,
        np.asarray(Wq), np.asarray(Wk), np.asarray(Wv), np.asarray(Wo),
    )
    res = bass_utils.run_bass_kernel_spmd(
        nc, in_maps, list(range(NCORES)), trace=_trace, **trace_kwargs
    )
    out = np.zeros((B, S, D), np.float32)
    for c in range(NCORES):
        out[c // HG] += res.results[c]["out"].astype(np.float32)
    if _want_results:
        return out, res
    return out
